# revision 51
# baseline (speedup 1.0000x reference)
# Causal self-attention (B=2, T=2048, D=1024, H=16, HD=64) with RoPE on 8 TRN2
# cores. Data-parallel over batch (2 groups of 4 cores), tensor-parallel over
# heads within a group (4 heads = 2 pairs per core).
#
# Schedule (engines balanced, PE kept dense):
#  - x arrives t-chunk-major so the qkv^T projection pipelines behind the
#    x load instead of stalling on it.
#  - q/k projected per head-pair into a [128,1024] PSUM tile (q|k halves),
#    RoPE'd with merged full-tile ops (DVE shuffle/cos/add + Pool sin) into a
#    merged bf16 q|k SBUF tile; v projected directly in [t, hd] layout (x
#    chunk stationary) from its own PSUM slot, so v matmuls fill the PE while
#    RoPE drains.
#  - attention strip si (512 q's) interleaves into the remaining projection
#    matmuls as PE filler: S^T (bf16) -> exp on the Scalar engine -> causal
#    triangle mask (one 3D affine_select on Pool) -> AV with an augmented
#    ones-column producing the softmax denominator. Strips 2-3 (71% of the
#    attention) get double-buffered S tiles so exp overlaps the next S.
#  - out-projection (row-sharded partial [D, T] per core) per strip, woven
#    into later strips' exp gaps; host sums 4 partials per batch.
#
# PSUM (8 banks): phase 1 = qk-proj slot (2) + v slot (2) + S (2) + AV (2);
# phase 2 (strips 2-3) = S x2 (4) + AV (2) + oproj (2).
import sys
import os

sys.path.insert(0, "/opt/trn_rl_repo")

import numpy as np
import ml_dtypes

import concourse.bass as bass  # noqa: F401
import concourse.mybir as mybir
from concourse import bacc
from concourse.tile import TileContext
from concourse.bass_utils import run_bass_kernel_spmd
from contextlib import ExitStack

F32 = mybir.dt.float32
F32R = mybir.dt.float32r
BF16 = mybir.dt.bfloat16
AF = mybir.ActivationFunctionType
ALU = mybir.AluOpType

B, T, D = 2, 2048, 1024
H, HD = 16, 64
NCORES = 8
GROUPS = NCORES // B          # cores per batch = 4
HPC = H // GROUPS             # heads per core = 4
NK = D // 128                 # contraction tiles = 8
NTC = T // 512                # t-chunks = 4
SCALE = HD ** -0.5

# hd interleave: new row 2j <- orig j, new row 2j+1 <- orig j+32 so the
# rotate-half partner of every row is its neighbour (swappable by a 32-lane
# stream shuffle).
PI = np.empty(HD, dtype=np.int64)
PI[0::2] = np.arange(32)
PI[1::2] = np.arange(32, 64)

SWAP_MASK = []
for _i in range(16):
    SWAP_MASK += [2 * _i + 1, 2 * _i]


def _build_program():
    nc = bacc.Bacc("TRN2", target_bir_lowering=False, debug=False,
                   num_devices=NCORES)
    d_xT = nc.dram_tensor("xT", [D, T], F32, kind="ExternalInput").ap()
    # cols: q01|k01|q23|k23 (PI-interleaved, 128 each) then v0..v3 (plain, 256)
    d_w = nc.dram_tensor("w_cat", [D, 6 * 128], F32, kind="ExternalInput").ap()
    d_wo = nc.dram_tensor("w_o", [2 * 128, D], BF16, kind="ExternalInput").ap()
    d_cos = nc.dram_tensor("cos2", [128, T], BF16, kind="ExternalInput").ap()
    d_sin = nc.dram_tensor("sin2", [128, T], BF16, kind="ExternalInput").ap()
    d_out = nc.dram_tensor("outp", [D, T], BF16, kind="ExternalOutput").ap()
    # pair-0 partial of the last strip's out-projection (host adds it in);
    # lets that oproj start before pair 1's attention finishes
    d_out2 = nc.dram_tensor("outp2", [D, 512], BF16,
                            kind="ExternalOutput").ap()
    # tiny scratch: holds the x tc1-3 loads back until the weights are in
    d_scr = nc.dram_tensor("scr", [1, 8], F32, kind="ExternalOutput").ap()
    dbg = bool(int(os.environ.get("KDEBUG", "0")))
    if dbg:
        d_dbg_qk0 = nc.dram_tensor("dbg_qk0", [128, 2 * T], BF16,
                                   kind="ExternalOutput").ap()
        d_dbg_va0 = nc.dram_tensor("dbg_va0", [128, 16 * 130], BF16,
                                   kind="ExternalOutput").ap()
        d_dbg_o0 = nc.dram_tensor("dbg_o0", [128, T], BF16,
                                  kind="ExternalOutput").ap()

    with TileContext(nc) as tc, nc.allow_low_precision(reason="bf16 attn"):
        with ExitStack() as root:
            xp = root.enter_context(tc.tile_pool(name="xp", bufs=1))
            wp = root.enter_context(tc.tile_pool(name="wp", bufs=1))
            tab = root.enter_context(tc.tile_pool(name="tab", bufs=1))
            qkp = root.enter_context(tc.tile_pool(name="qkp", bufs=1))
            vap_p = root.enter_context(tc.tile_pool(name="vap", bufs=1))
            otp = root.enter_context(tc.tile_pool(name="otp", bufs=1))
            wop = root.enter_context(tc.tile_pool(name="wop", bufs=1))
            rsc = root.enter_context(tc.tile_pool(name="rsc", bufs=2))
            ptp = root.enter_context(tc.tile_pool(name="ptp", bufs=6))
            rp = root.enter_context(tc.tile_pool(name="rp", bufs=2))
            fop = root.enter_context(tc.tile_pool(name="fop", bufs=4))

            x_sb = [xp.tile([128, T], F32R, tag=f"x{kt}", name=f"xsb{kt}")
                    for kt in range(NK)]
            w_sb = [wp.tile([128, 6 * 128], F32R, tag=f"w{kt}",
                            name=f"wsb{kt}") for kt in range(NK)]
            cos2 = tab.tile([128, T], BF16, tag="cos")
            sin2 = tab.tile([128, T], BF16, tag="sin")
            # merged q|k per pair: cols [0:T] = q, [T:2T] = k
            qk = [qkp.tile([128, 2 * T], BF16, tag=f"qk{p}", name=f"qk{p}")
                  for p in range(2)]
            # per pair: 16 k-blocks x [2 heads x (64 v | 1 ones)]
            vap = [vap_p.tile([128, 16 * 130], BF16, tag=f"va{p}",
                              name=f"vap{p}") for p in range(2)]
            oT = [otp.tile([128, T], BF16, tag=f"o{p}", name=f"oT{p}")
                  for p in range(2)]
            wo_sb = [wop.tile([128, D], BF16, tag=f"wo{p}", name=f"wo{p}")
                     for p in range(2)]

            # ---- DMA issue, ordered by need: all q|k weight columns first
            # (pace the tc0 kt loops for both pairs), RoPE tables, v weight
            # columns, wo last. x t-chunk 0 on the sync queue concurrently;
            # x tc1-3 held back behind a dummy DMA that depends on the last
            # v-weight slab, so they don't steal early bandwidth.
            for kt in range(NK):
                if kt == 4:
                    # tables land mid-load so RoPE starts the moment the
                    # first pair-tile completes
                    nc.scalar.dma_start(out=cos2[:], in_=d_cos[:])
                    nc.scalar.dma_start(out=sin2[:], in_=d_sin[:])
                nc.scalar.dma_start(
                    out=w_sb[kt][:, 0:512],
                    in_=d_w[kt * 128:(kt + 1) * 128, 0:512].bitcast(F32R))
            for kt in range(NK):
                nc.scalar.dma_start(
                    out=w_sb[kt][:, 512:768],
                    in_=d_w[kt * 128:(kt + 1) * 128, 512:768].bitcast(F32R))
            for p in range(2):
                nc.scalar.dma_start(
                    out=wo_sb[p][:], in_=d_wo[p * 128:(p + 1) * 128, :])
                # softmax-denominator ones columns
                nc.gpsimd.memset(vap[p][:, 64:16 * 130:65], 1.0)
            for kt in range(NK):
                nc.sync.dma_start(
                    out=x_sb[kt][:, 0:512],
                    in_=d_xT[kt * 128:(kt + 1) * 128, 0:512].bitcast(F32R))
            nc.sync.dma_start(out=d_scr[:],
                              in_=w_sb[7][0:1, 760:768].bitcast(F32))
            for tcc in range(1, NTC):
                for kt in range(NK):
                    nc.sync.dma_start(
                        out=x_sb[kt][:, tcc * 512:(tcc + 1) * 512],
                        in_=d_xT[kt * 128:(kt + 1) * 128,
                                 tcc * 512:(tcc + 1) * 512].bitcast(F32R))

            # ---- helpers ------------------------------------------------
            def mm_qk(t_ab, pr, tcc, kt):
                """pair pr's q into t_ab[:, 0:512], k into [:, 512:1024]."""
                c0 = tcc * 512
                for i, wc in ((0, 2 * pr), (1, 2 * pr + 1)):
                    nc.tensor.matmul(
                        t_ab[:, i * 512:(i + 1) * 512],
                        w_sb[kt][:, wc * 128:(wc + 1) * 128],
                        x_sb[kt][:, c0:c0 + 512],
                        start=(kt == 0), stop=(kt == NK - 1))

            def mm_v(t_v, tcc, kt):
                """v for 4 t-blocks: x chunk stationary, w_v moving.
                Two t-blocks share a PSUM bank and matmul start=True resets
                the whole bank, so only the even block starts (zeroing its
                bank) and the odd one accumulates from kt 0."""
                for tb in range(4):
                    t0 = tcc * 512 + tb * 128
                    nc.tensor.matmul(
                        t_v[:, tb * 256:(tb + 1) * 256],
                        x_sb[kt][:, t0:t0 + 128],
                        w_sb[kt][:, 4 * 128:6 * 128],
                        start=(kt == 0 and tb % 2 == 0),
                        stop=(kt == NK - 1),
                        skip_group_check=True)

            def emit_rope(ps_ab, p, tcc, which):
                """Drain a q|k PSUM pair-tile through RoPE into bf16 SBUF.
                Full-tile merged ops; sin-multiply on Pool (SBUF-only)."""
                cs = slice(tcc * 512, tcc * 512 + 512)
                src3 = ps_ab[:].rearrange("a (r t) -> a r t", r=2)
                cosd = cos2[:, cs].rearrange("a (r t) -> a r t",
                                             r=1).broadcast_to([128, 2, 512])
                sind = sin2[:, cs].rearrange("a (r t) -> a r t",
                                             r=1).broadcast_to([128, 2, 512])
                dst3 = qk[p][:].rearrange("a (r t) -> a r t", r=2)[:, :, cs]
                qsh = rsc.tile([128, 1024], F32, tag="qsh",
                               name=f"qsh{which}")
                tcs = rsc.tile([128, 1024], BF16, tag="tcs",
                               name=f"tcs{which}")
                qsm = rsc.tile([128, 1024], BF16, tag="qsm",
                               name=f"qsm{which}")
                qsh3 = qsh[:].rearrange("a (r t) -> a r t", r=2)
                tcs3 = tcs[:].rearrange("a (r t) -> a r t", r=2)
                qsm3 = qsm[:].rearrange("a (r t) -> a r t", r=2)
                nc.vector.stream_shuffle(qsh[:], ps_ab[:], SWAP_MASK)
                nc.vector.tensor_tensor(out=tcs3, in0=src3, in1=cosd,
                                        op=ALU.mult)
                nc.gpsimd.tensor_tensor(out=qsm3, in0=qsh3, in1=sind,
                                        op=ALU.mult)
                nc.vector.tensor_tensor(out=dst3, in0=qsm3, in1=tcs3,
                                        op=ALU.add)

            def emit_vdrain(ps_v, tcc):
                """PSUM [128, 4*(4*64)] -> vap strided (65-col head slots)."""
                v4 = ps_v[:].rearrange("a (tb h d) -> a tb h d", tb=4, h=4)
                for p in range(2):
                    o4 = vap[p][:].rearrange("a (kb hh e) -> a kb hh e",
                                             kb=16, hh=2)
                    nc.vector.tensor_copy(
                        out=o4[:, tcc * 4:(tcc + 1) * 4, :, 0:64],
                        in_=v4[:, :, 2 * p:2 * p + 2, :])

            # ---- PSUM phase 1 -------------------------------------------
            psA = tc.alloc_tile_pool(name="psA", bufs=1, space="PSUM")
            psC = tc.alloc_tile_pool(name="psC", bufs=1, space="PSUM")
            psS = tc.alloc_tile_pool(name="psS", bufs=1, space="PSUM",
                                     side="right")
            psV = tc.alloc_tile_pool(name="psV", bufs=1, space="PSUM",
                                     side="right")
            pools = {"S": psS, "V": psV}

            def emit_tc(tcc, fillable):
                """One t-chunk of projection: pair0 q|k, v (fills the RoPE
                drain window), pair1 q|k. Yields between PE work units."""
                t_ab = psA.tile([128, 1024], F32, tag="pa", name=f"pA{tcc}")
                for kt in range(NK):
                    mm_qk(t_ab, 0, tcc, kt)
                    if fillable:
                        yield
                emit_rope(t_ab, 0, tcc, f"A{tcc}")
                t_v = psC.tile([128, 1024], F32, tag="pc", name=f"pC{tcc}")
                for kt in range(NK):
                    mm_v(t_v, tcc, kt)
                    if fillable:
                        yield
                emit_vdrain(t_v, tcc)
                t_cd = psA.tile([128, 1024], F32, tag="pa", name=f"pB{tcc}")
                for kt in range(NK):
                    mm_qk(t_cd, 1, tcc, kt)
                    if fillable:
                        yield
                emit_rope(t_cd, 1, tcc, f"B{tcc}")

            # t-chunk 0 eagerly, special-cased: both q|k pairs project while
            # the load is DMA-paced (pair 1 borrows the v slot); v runs last,
            # overlapping the RoPE drains (its weights arrive last anyway).
            t_a0 = psA.tile([128, 1024], F32, tag="pa", name="pA0")
            t_b0 = psC.tile([128, 1024], F32, tag="pc", name="pB0")
            for kt in range(NK):
                mm_qk(t_a0, 0, 0, kt)
                mm_qk(t_b0, 1, 0, kt)
            emit_rope(t_a0, 0, 0, "A0")
            emit_rope(t_b0, 1, 0, "B0")
            t_v0 = psA.tile([128, 1024], F32, tag="pa", name="pC0")
            for kt in range(NK):
                mm_v(t_v0, 0, kt)
            emit_vdrain(t_v0, 0)

            def qkv_units():
                for tcc in range(1, NTC):
                    yield from emit_tc(tcc, True)

            state = {"gen": qkv_units()}

            def fill(n):
                g = state["gen"]
                if g is None:
                    return
                for _ in range(n):
                    try:
                        next(g)
                    except StopIteration:
                        state["gen"] = None
                        return

            def drain_fill():
                fill(1 << 30)

            # ---- out-projection ----------------------------------------
            def emit_oproj_n(psD, si, n, tail=False):
                q0 = si * 512
                pD = psD.tile([128, 512], F32, tag="pd", name=f"pD{si}_{n}")
                for p in range(2):
                    nc.tensor.matmul(
                        pD[:], wo_sb[p][:, n * 128:(n + 1) * 128],
                        oT[p][:, q0:q0 + 512],
                        start=(p == 0), stop=(p == 1))
                fo = fop.tile([128, 512], BF16, tag="fo", name=f"fo{si}_{n}")
                # DVE, not Act: exp pacing owns the Activation engine
                nc.vector.tensor_copy(out=fo[:], in_=pD[:])
                nc.sync.dma_start(
                    out=d_out[n * 128:(n + 1) * 128, q0:q0 + 512],
                    in_=fo[:])

            def oproj_units(psD, strips):
                # the final strip's oproj must NOT be a filler: it depends on
                # that strip's normalize, emitted after the fillers.
                for si in strips:
                    for n in range(8):
                        emit_oproj_n(psD, si, n)
                        yield

            def emit_oproj_p(psD, si, p, n, tail=False, tag="pd"):
                """Single-pair oproj partial: pair 0 -> d_out2 (host adds),
                pair 1 -> d_out. Lets pair 0's oproj overlap pair 1's
                attention on the final strip."""
                q0 = si * 512
                pD = psD.tile([128, 512], F32, tag=tag, name=f"pP{p}_{n}")
                nc.tensor.matmul(
                    pD[:], wo_sb[p][:, n * 128:(n + 1) * 128],
                    oT[p][:, q0:q0 + 512], start=True, stop=True)
                fo = fop.tile([128, 512], BF16, tag="fo", name=f"fp{p}_{n}")
                if tail and n % 2 == 0:
                    # Act is idle after the last exp; split the tail drain
                    nc.scalar.copy(fo[:], pD[:])
                else:
                    nc.vector.tensor_copy(out=fo[:], in_=pD[:])
                if p == 0:
                    nc.sync.dma_start(out=d_out2[n * 128:(n + 1) * 128, :],
                                      in_=fo[:])
                else:
                    nc.sync.dma_start(
                        out=d_out[n * 128:(n + 1) * 128, q0:q0 + 512],
                        in_=fo[:])

            def oproj_p_units(psD, si, p):
                for n in range(8):
                    emit_oproj_p(psD, si, p, n)
                    yield

            # ---- attention strip ----------------------------------------
            def emit_sexp(si, p, kb):
                """S^T matmuls for both heads of a pair + exp + causal mask.
                Returns what the (lagged) AV step needs."""
                q0 = 512 * si
                o = max(0, 128 * kb - q0)
                L = 512 - o
                sps = pools["S"].tile([128, 1024], F32, tag="ps",
                                      name=f"sps{si}_{p}_{kb}")
                for hl in range(2):
                    hb = 64 * hl
                    nc.tensor.matmul(
                        sps[:, 512 * hl + o:512 * hl + 512],
                        qk[p][hb:hb + 64, T + kb * 128:T + (kb + 1) * 128],
                        qk[p][hb:hb + 64, q0 + o:q0 + 512],
                        start=True, stop=True)
                ptb = ptp.tile([128, 1024], BF16, tag="ptb",
                               name=f"ptb{si}_{p}_{kb}")
                sps3 = sps[:].rearrange("a (h q) -> a h q", h=2)
                ptb3 = ptb[:].rearrange("a (h q) -> a h q", h=2)
                nc.scalar.activation(ptb3[:, :, 0:L], sps3[:, :, o:512],
                                     AF.Exp, scale=SCALE)
                if 128 * kb >= q0:
                    # diagonal block: zero cols j < partition (q < k),
                    # both heads in one 3D affine_select
                    nc.gpsimd.affine_select(
                        ptb3[:, :, 0:128], ptb3[:, :, 0:128],
                        pattern=[[0, 2], [1, 128]],
                        compare_op=ALU.is_ge, fill=0.0, base=0,
                        channel_multiplier=-1)
                return (ptb, o, L, kb)

            def emit_av(si, p, av, work, hls=(0, 1)):
                ptb, o, L, kb = work
                kbmax = 4 * (si + 1)
                for hl in hls:
                    nc.tensor.matmul(
                        av[hl][:, o:512],
                        vap[p][:, kb * 130 + hl * 65:kb * 130 + hl * 65 + 65],
                        ptb[:, 512 * hl:512 * hl + L],
                        start=(kb == 0), stop=(kb == kbmax - 1),
                        skip_group_check=True)

            def emit_norm(si, p, av, hl):
                q0 = 512 * si
                r_sb = rp.tile([1, 512], F32, tag="r",
                               name=f"rsb{si}_{p}_{hl}")
                nc.vector.reciprocal(r_sb[:], av[hl][64:65, :])
                rb = rp.tile([64, 512], F32, tag="rb",
                             name=f"rbb{si}_{p}_{hl}")
                nc.gpsimd.partition_broadcast(rb[:], r_sb[:])
                nc.vector.tensor_tensor(
                    out=oT[p][64 * hl:64 * hl + 64, q0:q0 + 512],
                    in0=av[hl][0:64, :], in1=rb[:], op=ALU.mult)

            def emit_strip_pair(si, p, nfill, pre=None):
                """One pair, serial; AV lags S by one k-block so it never
                waits on a just-issued exp. `pre` = kb-0 work emitted by the
                caller ahead of a pool re-plumb barrier."""
                q0 = 512 * si
                kbmax = 4 * (si + 1)
                av = [pools["V"].tile([65, 512], F32, tag=f"av{hl}",
                                      name=f"av{si}_{p}_{hl}")
                      for hl in range(2)]
                prev = pre
                for kb in range(1 if pre is not None else 0, kbmax):
                    work = emit_sexp(si, p, kb)
                    fill(nfill)
                    if prev is not None:
                        emit_av(si, p, av, prev)
                    fill(nfill)
                    prev = work
                # last k-block: AV and normalize interleaved per head so the
                # tail normalize overlaps the other head's AV
                for hl in range(2):
                    emit_av(si, p, av, prev, hls=(hl,))
                    emit_norm(si, p, av, hl)

            def emit_strip_both(si, pre=None):
                """Both pairs interleaved per k-block, AV lagging one
                k-block: two exp streams keep the Activation engine
                saturated while the PE never waits on a fresh exp. Needs 4
                AV banks + 2 S slots (shared between the pairs). `pre` =
                kb-0 work per pair emitted ahead of the pool barrier."""
                q0 = 512 * si
                kbmax = 4 * (si + 1)
                av = {p: [pools["V"].tile([65, 512], F32, tag=f"av{hl}",
                                          name=f"av{si}_{p}_{hl}")
                          for hl in range(2)] for p in range(2)}
                prev = dict(pre) if pre else {0: None, 1: None}
                for kb in range(1 if pre else 0, kbmax):
                    for p in range(2):
                        work = emit_sexp(si, p, kb)
                        if prev[p] is not None:
                            emit_av(si, p, av[p], prev[p])
                        prev[p] = work
                for p in range(2):
                    for hl in range(2):
                        emit_av(si, p, av[p], prev[p], hls=(hl,))
                        emit_norm(si, p, av[p], hl)

            with nc.named_scope("attn"):
                for si in range(2):
                    emit_strip_pair(si, 0, 2)
                    emit_strip_pair(si, 1, 2)
                drain_fill()
                # re-plumb PSUM: double-buffered S (exp overlaps next S) +
                # both pairs' AV accumulators — strip 3 runs pairs-interleaved
                # (kb-0 S/exp pre-emitted so the PE works through the barrier)
                w3 = {p: emit_sexp(3, p, 0) for p in range(2)}
                psV.release()
                psS.release()
                psC.release()
                psA.release()
                psS2 = tc.alloc_tile_pool(name="psS2", bufs=2, space="PSUM")
                psV2 = tc.alloc_tile_pool(name="psV2", bufs=2, space="PSUM",
                                          side="right")
                pools["S"], pools["V"] = psS2, psV2
                emit_strip_both(3, pre=w3)
                # strip 2 last (smallest tail): pairs serial, S still
                # double-buffered, with oproj fillers for strips 0/1/3 and
                # then the pair-split oproj of strip 2 itself
                psV2.release()
                w20 = emit_sexp(2, 0, 0)
                psV3 = tc.alloc_tile_pool(name="psV3", bufs=1, space="PSUM",
                                          side="right")
                psD = tc.alloc_tile_pool(name="psD", bufs=2, space="PSUM",
                                         side="right")
                pools["V"] = psV3
                state["gen"] = oproj_units(psD, [0, 1, 3])
                emit_strip_pair(2, 0, 1, pre=w20)
                drain_fill()
                state["gen"] = oproj_p_units(psD, 2, 0)
                emit_strip_pair(2, 1, 1)
                drain_fill()
                for n in range(8):
                    # alternate between psD and the now-idle S slots for a
                    # 4-deep accumulator rotation in the drain-limited tail
                    if n % 2:
                        emit_oproj_p(psS2, 2, 1, n, tail=True, tag="ps")
                    else:
                        emit_oproj_p(psD, 2, 1, n, tail=True)
                psD.release()
                psV3.release()
                psS2.release()

            if dbg:
                nc.sync.dma_start(out=d_dbg_qk0[:], in_=qk[0][:])
                nc.sync.dma_start(out=d_dbg_va0[:], in_=vap[0][:])
                nc.sync.dma_start(out=d_dbg_o0[:], in_=oT[0][:])

    nc.compile()
    return nc


_NC_CACHE = None


def _get_program():
    global _NC_CACHE
    if _NC_CACHE is None:
        _NC_CACHE = _build_program()
    return _NC_CACHE


def _rope_tables():
    inv_freq = 1.0 / (10000.0 ** (np.arange(0, HD, 2, dtype=np.float32) / HD))
    freqs = np.outer(np.arange(T, dtype=np.float32), inv_freq)  # [T, 32]
    emb = np.concatenate([freqs, freqs], axis=-1)               # [T, 64]
    return np.cos(emb), np.sin(emb)


def _host_prep(x, w_qkv, w_out):
    bf = ml_dtypes.bfloat16
    cos, sin = _rope_tables()          # [T, 64], original hd order
    cosP = np.ascontiguousarray(cos.T[PI, :])                   # [64, T]
    sinP = sin.T[PI, :].copy()                                  # [64, T]
    sinP[0::2, :] *= -1.0                                       # sign baked in
    cos2d = np.ascontiguousarray(np.vstack([cosP, cosP])).astype(bf)
    sin2d = np.ascontiguousarray(np.vstack([sinP, sinP])).astype(bf)

    in_maps = []
    for core in range(NCORES):
        b = core // GROUPS
        h0 = (core % GROUPS) * HPC
        xT = np.ascontiguousarray(x[b].T)                       # [D, T]
        qc, kc = [], []
        for pr in range(2):
            for hh in range(2):
                h = h0 + 2 * pr + hh
                qc.append(w_qkv[:, h * HD:(h + 1) * HD][:, PI])
                kc.append(w_qkv[:, D + h * HD:D + (h + 1) * HD][:, PI])
        cols = [qc[0], qc[1], kc[0], kc[1], qc[2], qc[3], kc[2], kc[3],
                w_qkv[:, 2 * D + h0 * HD:2 * D + (h0 + HPC) * HD]]
        w_cat = np.ascontiguousarray(np.concatenate(cols, axis=1),
                                     dtype=np.float32)          # [D, 768]
        w_o = np.ascontiguousarray(
            w_out[h0 * HD:(h0 + HPC) * HD, :]).astype(bf)       # [256, D]
        in_maps.append({
            "xT": xT.astype(np.float32, copy=False),
            "w_cat": w_cat,
            "w_o": w_o,
            "cos2": cos2d,
            "sin2": sin2d,
        })
    return in_maps


def kernel(x, w_qkv, w_out):
    x = np.asarray(x, dtype=np.float32)
    w_qkv = np.asarray(w_qkv, dtype=np.float32)
    w_out = np.asarray(w_out, dtype=np.float32)
    nc = _get_program()
    in_maps = _host_prep(x, w_qkv, w_out)
    res = run_bass_kernel_spmd(nc, in_maps, list(range(NCORES)), trace=False)
    out = np.zeros((B, T, D), dtype=np.float32)
    for core in range(NCORES):
        b = core // GROUPS
        out[b] += res.results[core]["outp"].T.astype(np.float32)
        # strip-2 pair-0 oproj partial shipped separately
        out[b][2 * 512:3 * 512] += \
            res.results[core]["outp2"].T.astype(np.float32)
    return out


# revision 52
# speedup vs baseline: 1.0160x; 1.0160x over previous
# Causal self-attention (B=2, T=2048, D=1024, H=16, HD=64) with RoPE on 8 TRN2
# cores. Data-parallel over batch (2 groups of 4 cores), tensor-parallel over
# heads within a group (4 heads = 2 pairs per core).
#
# Schedule (engines balanced, PE kept dense):
#  - x arrives t-chunk-major so the qkv^T projection pipelines behind the
#    x load instead of stalling on it.
#  - q/k projected per head-pair into a [128,1024] PSUM tile (q|k halves),
#    RoPE'd with merged full-tile ops (DVE shuffle/cos/add + Pool sin) into a
#    merged bf16 q|k SBUF tile; v projected directly in [t, hd] layout (x
#    chunk stationary) from its own PSUM slot, so v matmuls fill the PE while
#    RoPE drains.
#  - attention strip si (512 q's) interleaves into the remaining projection
#    matmuls as PE filler: S^T (bf16) -> exp on the Scalar engine -> causal
#    triangle mask (one 3D affine_select on Pool) -> AV with an augmented
#    ones-column producing the softmax denominator. Strips 2-3 (71% of the
#    attention) get double-buffered S tiles so exp overlaps the next S.
#  - out-projection (row-sharded partial [D, T] per core) per strip, woven
#    into later strips' exp gaps; host sums 4 partials per batch.
#
# PSUM (8 banks): phase 1 = qk-proj slot (2) + v slot (2) + S (2) + AV (2);
# phase 2 (strips 2-3) = S x2 (4) + AV (2) + oproj (2).
import sys
import os

sys.path.insert(0, "/opt/trn_rl_repo")

import numpy as np
import ml_dtypes

import concourse.bass as bass  # noqa: F401
import concourse.mybir as mybir
from concourse import bacc
from concourse.tile import TileContext
from concourse.bass_utils import run_bass_kernel_spmd
from contextlib import ExitStack

F32 = mybir.dt.float32
F32R = mybir.dt.float32r
BF16 = mybir.dt.bfloat16
AF = mybir.ActivationFunctionType
ALU = mybir.AluOpType

B, T, D = 2, 2048, 1024
H, HD = 16, 64
NCORES = 8
GROUPS = NCORES // B          # cores per batch = 4
HPC = H // GROUPS             # heads per core = 4
NK = D // 128                 # contraction tiles = 8
NTC = T // 512                # t-chunks = 4
SCALE = HD ** -0.5

# hd interleave: new row 2j <- orig j, new row 2j+1 <- orig j+32 so the
# rotate-half partner of every row is its neighbour (swappable by a 32-lane
# stream shuffle).
PI = np.empty(HD, dtype=np.int64)
PI[0::2] = np.arange(32)
PI[1::2] = np.arange(32, 64)

SWAP_MASK = []
for _i in range(16):
    SWAP_MASK += [2 * _i + 1, 2 * _i]


def _build_program():
    nc = bacc.Bacc("TRN2", target_bir_lowering=False, debug=False,
                   num_devices=NCORES)
    d_xT = nc.dram_tensor("xT", [D, T], F32, kind="ExternalInput").ap()
    # cols: q01|k01|q23|k23 (PI-interleaved, 128 each) then v0..v3 (plain, 256)
    d_w = nc.dram_tensor("w_cat", [D, 6 * 128], F32, kind="ExternalInput").ap()
    d_wo = nc.dram_tensor("w_o", [2 * 128, D], BF16, kind="ExternalInput").ap()
    d_cos = nc.dram_tensor("cos2", [128, T], BF16, kind="ExternalInput").ap()
    d_sin = nc.dram_tensor("sin2", [128, T], BF16, kind="ExternalInput").ap()
    d_out = nc.dram_tensor("outp", [D, T], BF16, kind="ExternalOutput").ap()
    # pair-0 partial of the last strip's out-projection (host adds it in);
    # lets that oproj start before pair 1's attention finishes
    d_out2 = nc.dram_tensor("outp2", [D, 512], BF16,
                            kind="ExternalOutput").ap()
    # tiny scratch: holds the x tc1-3 loads back until the weights are in
    d_scr = nc.dram_tensor("scr", [1, 8], F32, kind="ExternalOutput").ap()
    dbg = bool(int(os.environ.get("KDEBUG", "0")))
    if dbg:
        d_dbg_qk0 = nc.dram_tensor("dbg_qk0", [128, 2 * T], BF16,
                                   kind="ExternalOutput").ap()
        d_dbg_va0 = nc.dram_tensor("dbg_va0", [128, 16 * 130], BF16,
                                   kind="ExternalOutput").ap()
        d_dbg_o0 = nc.dram_tensor("dbg_o0", [128, T], BF16,
                                  kind="ExternalOutput").ap()

    with TileContext(nc) as tc, nc.allow_low_precision(reason="bf16 attn"):
        with ExitStack() as root:
            xp = root.enter_context(tc.tile_pool(name="xp", bufs=1))
            wp = root.enter_context(tc.tile_pool(name="wp", bufs=1))
            tab = root.enter_context(tc.tile_pool(name="tab", bufs=1))
            qkp = root.enter_context(tc.tile_pool(name="qkp", bufs=1))
            vap_p = root.enter_context(tc.tile_pool(name="vap", bufs=1))
            otp = root.enter_context(tc.tile_pool(name="otp", bufs=1))
            wop = root.enter_context(tc.tile_pool(name="wop", bufs=1))
            rsc = root.enter_context(tc.tile_pool(name="rsc", bufs=2))
            ptp = root.enter_context(tc.tile_pool(name="ptp", bufs=6))
            rp = root.enter_context(tc.tile_pool(name="rp", bufs=2))
            fop = root.enter_context(tc.tile_pool(name="fop", bufs=4))

            x_sb = [xp.tile([128, T], F32R, tag=f"x{kt}", name=f"xsb{kt}")
                    for kt in range(NK)]
            w_sb = [wp.tile([128, 6 * 128], F32R, tag=f"w{kt}",
                            name=f"wsb{kt}") for kt in range(NK)]
            cos2 = tab.tile([128, T], BF16, tag="cos")
            sin2 = tab.tile([128, T], BF16, tag="sin")
            # merged q|k per pair: cols [0:T] = q, [T:2T] = k
            qk = [qkp.tile([128, 2 * T], BF16, tag=f"qk{p}", name=f"qk{p}")
                  for p in range(2)]
            # per pair: 16 k-blocks x [2 heads x (64 v | 1 ones)]
            vap = [vap_p.tile([128, 16 * 130], BF16, tag=f"va{p}",
                              name=f"vap{p}") for p in range(2)]
            oT = [otp.tile([128, T], BF16, tag=f"o{p}", name=f"oT{p}")
                  for p in range(2)]
            wo_sb = [wop.tile([128, D], BF16, tag=f"wo{p}", name=f"wo{p}")
                     for p in range(2)]

            # ---- DMA issue, ordered by need: all q|k weight columns first
            # (pace the tc0 kt loops for both pairs), RoPE tables, v weight
            # columns, wo last. x t-chunk 0 on the sync queue concurrently;
            # x tc1-3 held back behind a dummy DMA that depends on the last
            # v-weight slab, so they don't steal early bandwidth.
            for kt in range(NK):
                nc.scalar.dma_start(
                    out=w_sb[kt][:, 0:512],
                    in_=d_w[kt * 128:(kt + 1) * 128, 0:512].bitcast(F32R))
            nc.scalar.dma_start(out=cos2[:], in_=d_cos[:])
            nc.scalar.dma_start(out=sin2[:], in_=d_sin[:])
            for kt in range(NK):
                nc.scalar.dma_start(
                    out=w_sb[kt][:, 512:768],
                    in_=d_w[kt * 128:(kt + 1) * 128, 512:768].bitcast(F32R))
            for p in range(2):
                nc.scalar.dma_start(
                    out=wo_sb[p][:], in_=d_wo[p * 128:(p + 1) * 128, :])
                # softmax-denominator ones columns
                nc.gpsimd.memset(vap[p][:, 64:16 * 130:65], 1.0)
            for kt in range(NK):
                nc.sync.dma_start(
                    out=x_sb[kt][:, 0:512],
                    in_=d_xT[kt * 128:(kt + 1) * 128, 0:512].bitcast(F32R))
            nc.sync.dma_start(out=d_scr[:],
                              in_=w_sb[7][0:1, 760:768].bitcast(F32))
            for tcc in range(1, NTC):
                for kt in range(NK):
                    nc.sync.dma_start(
                        out=x_sb[kt][:, tcc * 512:(tcc + 1) * 512],
                        in_=d_xT[kt * 128:(kt + 1) * 128,
                                 tcc * 512:(tcc + 1) * 512].bitcast(F32R))

            # ---- helpers ------------------------------------------------
            def mm_qk(t_ab, pr, tcc, kt):
                """pair pr's q into t_ab[:, 0:512], k into [:, 512:1024]."""
                c0 = tcc * 512
                for i, wc in ((0, 2 * pr), (1, 2 * pr + 1)):
                    nc.tensor.matmul(
                        t_ab[:, i * 512:(i + 1) * 512],
                        w_sb[kt][:, wc * 128:(wc + 1) * 128],
                        x_sb[kt][:, c0:c0 + 512],
                        start=(kt == 0), stop=(kt == NK - 1))

            def mm_v(t_v, tcc, kt):
                """v for 4 t-blocks: x chunk stationary, w_v moving.
                Two t-blocks share a PSUM bank and matmul start=True resets
                the whole bank, so only the even block starts (zeroing its
                bank) and the odd one accumulates from kt 0."""
                for tb in range(4):
                    t0 = tcc * 512 + tb * 128
                    nc.tensor.matmul(
                        t_v[:, tb * 256:(tb + 1) * 256],
                        x_sb[kt][:, t0:t0 + 128],
                        w_sb[kt][:, 4 * 128:6 * 128],
                        start=(kt == 0 and tb % 2 == 0),
                        stop=(kt == NK - 1),
                        skip_group_check=True)

            def emit_rope(ps_ab, p, tcc, which):
                """Drain a q|k PSUM pair-tile through RoPE into bf16 SBUF.
                Full-tile merged ops; sin-multiply on Pool (SBUF-only)."""
                cs = slice(tcc * 512, tcc * 512 + 512)
                src3 = ps_ab[:].rearrange("a (r t) -> a r t", r=2)
                cosd = cos2[:, cs].rearrange("a (r t) -> a r t",
                                             r=1).broadcast_to([128, 2, 512])
                sind = sin2[:, cs].rearrange("a (r t) -> a r t",
                                             r=1).broadcast_to([128, 2, 512])
                dst3 = qk[p][:].rearrange("a (r t) -> a r t", r=2)[:, :, cs]
                qsh = rsc.tile([128, 1024], F32, tag="qsh",
                               name=f"qsh{which}")
                tcs = rsc.tile([128, 1024], BF16, tag="tcs",
                               name=f"tcs{which}")
                qsm = rsc.tile([128, 1024], BF16, tag="qsm",
                               name=f"qsm{which}")
                qsh3 = qsh[:].rearrange("a (r t) -> a r t", r=2)
                tcs3 = tcs[:].rearrange("a (r t) -> a r t", r=2)
                qsm3 = qsm[:].rearrange("a (r t) -> a r t", r=2)
                nc.vector.stream_shuffle(qsh[:], ps_ab[:], SWAP_MASK)
                nc.vector.tensor_tensor(out=tcs3, in0=src3, in1=cosd,
                                        op=ALU.mult)
                nc.gpsimd.tensor_tensor(out=qsm3, in0=qsh3, in1=sind,
                                        op=ALU.mult)
                nc.vector.tensor_tensor(out=dst3, in0=qsm3, in1=tcs3,
                                        op=ALU.add)

            def emit_vdrain(ps_v, tcc):
                """PSUM [128, 4*(4*64)] -> vap strided (65-col head slots)."""
                v4 = ps_v[:].rearrange("a (tb h d) -> a tb h d", tb=4, h=4)
                for p in range(2):
                    o4 = vap[p][:].rearrange("a (kb hh e) -> a kb hh e",
                                             kb=16, hh=2)
                    nc.vector.tensor_copy(
                        out=o4[:, tcc * 4:(tcc + 1) * 4, :, 0:64],
                        in_=v4[:, :, 2 * p:2 * p + 2, :])

            # ---- PSUM phase 1 -------------------------------------------
            psA = tc.alloc_tile_pool(name="psA", bufs=1, space="PSUM")
            psC = tc.alloc_tile_pool(name="psC", bufs=1, space="PSUM")
            psS = tc.alloc_tile_pool(name="psS", bufs=1, space="PSUM",
                                     side="right")
            psV = tc.alloc_tile_pool(name="psV", bufs=1, space="PSUM",
                                     side="right")
            pools = {"S": psS, "V": psV}

            def emit_tc(tcc, fillable):
                """One t-chunk of projection: pair0 q|k, v (fills the RoPE
                drain window), pair1 q|k. Yields between PE work units."""
                t_ab = psA.tile([128, 1024], F32, tag="pa", name=f"pA{tcc}")
                for kt in range(NK):
                    mm_qk(t_ab, 0, tcc, kt)
                    if fillable:
                        yield
                emit_rope(t_ab, 0, tcc, f"A{tcc}")
                t_v = psC.tile([128, 1024], F32, tag="pc", name=f"pC{tcc}")
                for kt in range(NK):
                    mm_v(t_v, tcc, kt)
                    if fillable:
                        yield
                emit_vdrain(t_v, tcc)
                t_cd = psA.tile([128, 1024], F32, tag="pa", name=f"pB{tcc}")
                for kt in range(NK):
                    mm_qk(t_cd, 1, tcc, kt)
                    if fillable:
                        yield
                emit_rope(t_cd, 1, tcc, f"B{tcc}")

            # t-chunk 0 eagerly, special-cased: both q|k pairs project while
            # the load is DMA-paced (pair 1 borrows the v slot); v runs last,
            # overlapping the RoPE drains (its weights arrive last anyway).
            t_a0 = psA.tile([128, 1024], F32, tag="pa", name="pA0")
            t_b0 = psC.tile([128, 1024], F32, tag="pc", name="pB0")
            for kt in range(NK):
                mm_qk(t_a0, 0, 0, kt)
                mm_qk(t_b0, 1, 0, kt)
            emit_rope(t_a0, 0, 0, "A0")
            emit_rope(t_b0, 1, 0, "B0")
            t_v0 = psA.tile([128, 1024], F32, tag="pa", name="pC0")
            for kt in range(NK):
                mm_v(t_v0, 0, kt)
            emit_vdrain(t_v0, 0)

            def qkv_units():
                for tcc in range(1, NTC):
                    yield from emit_tc(tcc, True)

            state = {"gen": qkv_units()}

            def fill(n):
                g = state["gen"]
                if g is None:
                    return
                for _ in range(n):
                    try:
                        next(g)
                    except StopIteration:
                        state["gen"] = None
                        return

            def drain_fill():
                fill(1 << 30)

            # ---- out-projection ----------------------------------------
            def emit_oproj_n(psD, si, n, tail=False):
                q0 = si * 512
                pD = psD.tile([128, 512], F32, tag="pd", name=f"pD{si}_{n}")
                for p in range(2):
                    nc.tensor.matmul(
                        pD[:], wo_sb[p][:, n * 128:(n + 1) * 128],
                        oT[p][:, q0:q0 + 512],
                        start=(p == 0), stop=(p == 1))
                fo = fop.tile([128, 512], BF16, tag="fo", name=f"fo{si}_{n}")
                # DVE, not Act: exp pacing owns the Activation engine
                nc.vector.tensor_copy(out=fo[:], in_=pD[:])
                nc.sync.dma_start(
                    out=d_out[n * 128:(n + 1) * 128, q0:q0 + 512],
                    in_=fo[:])

            def oproj_units(psD, strips):
                # the final strip's oproj must NOT be a filler: it depends on
                # that strip's normalize, emitted after the fillers.
                for si in strips:
                    for n in range(8):
                        emit_oproj_n(psD, si, n)
                        yield

            def emit_oproj_p(psD, si, p, n, tail=False, tag="pd"):
                """Single-pair oproj partial: pair 0 -> d_out2 (host adds),
                pair 1 -> d_out. Lets pair 0's oproj overlap pair 1's
                attention on the final strip."""
                q0 = si * 512
                pD = psD.tile([128, 512], F32, tag=tag, name=f"pP{p}_{n}")
                nc.tensor.matmul(
                    pD[:], wo_sb[p][:, n * 128:(n + 1) * 128],
                    oT[p][:, q0:q0 + 512], start=True, stop=True)
                fo = fop.tile([128, 512], BF16, tag="fo", name=f"fp{p}_{n}")
                if tail and n % 2 == 0:
                    # Act is idle after the last exp; split the tail drain
                    nc.scalar.copy(fo[:], pD[:])
                else:
                    nc.vector.tensor_copy(out=fo[:], in_=pD[:])
                if p == 0:
                    nc.sync.dma_start(out=d_out2[n * 128:(n + 1) * 128, :],
                                      in_=fo[:])
                else:
                    nc.sync.dma_start(
                        out=d_out[n * 128:(n + 1) * 128, q0:q0 + 512],
                        in_=fo[:])

            def oproj_p_units(psD, si, p):
                for n in range(8):
                    emit_oproj_p(psD, si, p, n)
                    yield

            # ---- attention strip ----------------------------------------
            def emit_sexp(si, p, kb):
                """S^T matmuls for both heads of a pair + exp + causal mask.
                Returns what the (lagged) AV step needs."""
                q0 = 512 * si
                o = max(0, 128 * kb - q0)
                L = 512 - o
                sps = pools["S"].tile([128, 1024], F32, tag="ps",
                                      name=f"sps{si}_{p}_{kb}")
                for hl in range(2):
                    hb = 64 * hl
                    nc.tensor.matmul(
                        sps[:, 512 * hl + o:512 * hl + 512],
                        qk[p][hb:hb + 64, T + kb * 128:T + (kb + 1) * 128],
                        qk[p][hb:hb + 64, q0 + o:q0 + 512],
                        start=True, stop=True)
                ptb = ptp.tile([128, 1024], BF16, tag="ptb",
                               name=f"ptb{si}_{p}_{kb}")
                sps3 = sps[:].rearrange("a (h q) -> a h q", h=2)
                ptb3 = ptb[:].rearrange("a (h q) -> a h q", h=2)
                nc.scalar.activation(ptb3[:, :, 0:L], sps3[:, :, o:512],
                                     AF.Exp, scale=SCALE)
                if 128 * kb >= q0:
                    # diagonal block: zero cols j < partition (q < k),
                    # both heads in one 3D affine_select
                    nc.gpsimd.affine_select(
                        ptb3[:, :, 0:128], ptb3[:, :, 0:128],
                        pattern=[[0, 2], [1, 128]],
                        compare_op=ALU.is_ge, fill=0.0, base=0,
                        channel_multiplier=-1)
                return (ptb, o, L, kb)

            def emit_av(si, p, av, work, hls=(0, 1)):
                ptb, o, L, kb = work
                kbmax = 4 * (si + 1)
                for hl in hls:
                    nc.tensor.matmul(
                        av[hl][:, o:512],
                        vap[p][:, kb * 130 + hl * 65:kb * 130 + hl * 65 + 65],
                        ptb[:, 512 * hl:512 * hl + L],
                        start=(kb == 0), stop=(kb == kbmax - 1),
                        skip_group_check=True)

            def emit_norm(si, p, av, hl):
                q0 = 512 * si
                r_sb = rp.tile([1, 512], F32, tag="r",
                               name=f"rsb{si}_{p}_{hl}")
                nc.vector.reciprocal(r_sb[:], av[hl][64:65, :])
                rb = rp.tile([64, 512], F32, tag="rb",
                             name=f"rbb{si}_{p}_{hl}")
                nc.gpsimd.partition_broadcast(rb[:], r_sb[:])
                nc.vector.tensor_tensor(
                    out=oT[p][64 * hl:64 * hl + 64, q0:q0 + 512],
                    in0=av[hl][0:64, :], in1=rb[:], op=ALU.mult)

            def emit_strip_pair(si, p, nfill, pre=None):
                """One pair, serial; AV lags S by one k-block so it never
                waits on a just-issued exp. `pre` = kb-0 work emitted by the
                caller ahead of a pool re-plumb barrier."""
                q0 = 512 * si
                kbmax = 4 * (si + 1)
                av = [pools["V"].tile([65, 512], F32, tag=f"av{hl}",
                                      name=f"av{si}_{p}_{hl}")
                      for hl in range(2)]
                prev = pre
                for kb in range(1 if pre is not None else 0, kbmax):
                    work = emit_sexp(si, p, kb)
                    fill(nfill)
                    if prev is not None:
                        emit_av(si, p, av, prev)
                    fill(nfill)
                    prev = work
                # last k-block: AV and normalize interleaved per head so the
                # tail normalize overlaps the other head's AV
                for hl in range(2):
                    emit_av(si, p, av, prev, hls=(hl,))
                    emit_norm(si, p, av, hl)

            def emit_strip_both(si, pre=None):
                """Both pairs interleaved per k-block, AV lagging one
                k-block: two exp streams keep the Activation engine
                saturated while the PE never waits on a fresh exp. Needs 4
                AV banks + 2 S slots (shared between the pairs). `pre` =
                kb-0 work per pair emitted ahead of the pool barrier."""
                q0 = 512 * si
                kbmax = 4 * (si + 1)
                av = {p: [pools["V"].tile([65, 512], F32, tag=f"av{hl}",
                                          name=f"av{si}_{p}_{hl}")
                          for hl in range(2)] for p in range(2)}
                prev = dict(pre) if pre else {0: None, 1: None}
                for kb in range(1 if pre else 0, kbmax):
                    for p in range(2):
                        work = emit_sexp(si, p, kb)
                        if prev[p] is not None:
                            emit_av(si, p, av[p], prev[p])
                        prev[p] = work
                for p in range(2):
                    for hl in range(2):
                        emit_av(si, p, av[p], prev[p], hls=(hl,))
                        emit_norm(si, p, av[p], hl)

            with nc.named_scope("attn"):
                for si in range(2):
                    emit_strip_pair(si, 0, 2)
                    emit_strip_pair(si, 1, 2)
                drain_fill()
                # re-plumb PSUM: double-buffered S (exp overlaps next S) +
                # both pairs' AV accumulators — strip 3 runs pairs-interleaved
                # (kb-0 S/exp pre-emitted so the PE works through the barrier)
                w3 = {p: emit_sexp(3, p, 0) for p in range(2)}
                psV.release()
                psS.release()
                psC.release()
                psA.release()
                psS2 = tc.alloc_tile_pool(name="psS2", bufs=2, space="PSUM")
                psV2 = tc.alloc_tile_pool(name="psV2", bufs=2, space="PSUM",
                                          side="right")
                pools["S"], pools["V"] = psS2, psV2
                emit_strip_both(3, pre=w3)
                # strip 2 last (smallest tail): pairs serial, S still
                # double-buffered, with oproj fillers for strips 0/1/3 and
                # then the pair-split oproj of strip 2 itself
                psV2.release()
                w20 = emit_sexp(2, 0, 0)
                psV3 = tc.alloc_tile_pool(name="psV3", bufs=1, space="PSUM",
                                          side="right")
                psD = tc.alloc_tile_pool(name="psD", bufs=2, space="PSUM",
                                         side="right")
                pools["V"] = psV3
                state["gen"] = oproj_units(psD, [0, 1, 3])
                emit_strip_pair(2, 0, 1, pre=w20)
                drain_fill()
                state["gen"] = oproj_p_units(psD, 2, 0)
                emit_strip_pair(2, 1, 1)
                drain_fill()
                for n in range(8):
                    # alternate between psD and the now-idle S slots for a
                    # 4-deep accumulator rotation in the drain-limited tail
                    if n % 2:
                        emit_oproj_p(psS2, 2, 1, n, tail=True, tag="ps")
                    else:
                        emit_oproj_p(psD, 2, 1, n, tail=True)
                psD.release()
                psV3.release()
                psS2.release()

            if dbg:
                nc.sync.dma_start(out=d_dbg_qk0[:], in_=qk[0][:])
                nc.sync.dma_start(out=d_dbg_va0[:], in_=vap[0][:])
                nc.sync.dma_start(out=d_dbg_o0[:], in_=oT[0][:])

    nc.compile()
    return nc


_NC_CACHE = None


def _get_program():
    global _NC_CACHE
    if _NC_CACHE is None:
        _NC_CACHE = _build_program()
    return _NC_CACHE


def _rope_tables():
    inv_freq = 1.0 / (10000.0 ** (np.arange(0, HD, 2, dtype=np.float32) / HD))
    freqs = np.outer(np.arange(T, dtype=np.float32), inv_freq)  # [T, 32]
    emb = np.concatenate([freqs, freqs], axis=-1)               # [T, 64]
    return np.cos(emb), np.sin(emb)


def _host_prep(x, w_qkv, w_out):
    bf = ml_dtypes.bfloat16
    cos, sin = _rope_tables()          # [T, 64], original hd order
    cosP = np.ascontiguousarray(cos.T[PI, :])                   # [64, T]
    sinP = sin.T[PI, :].copy()                                  # [64, T]
    sinP[0::2, :] *= -1.0                                       # sign baked in
    cos2d = np.ascontiguousarray(np.vstack([cosP, cosP])).astype(bf)
    sin2d = np.ascontiguousarray(np.vstack([sinP, sinP])).astype(bf)

    in_maps = []
    for core in range(NCORES):
        b = core // GROUPS
        h0 = (core % GROUPS) * HPC
        xT = np.ascontiguousarray(x[b].T)                       # [D, T]
        qc, kc = [], []
        for pr in range(2):
            for hh in range(2):
                h = h0 + 2 * pr + hh
                qc.append(w_qkv[:, h * HD:(h + 1) * HD][:, PI])
                kc.append(w_qkv[:, D + h * HD:D + (h + 1) * HD][:, PI])
        cols = [qc[0], qc[1], kc[0], kc[1], qc[2], qc[3], kc[2], kc[3],
                w_qkv[:, 2 * D + h0 * HD:2 * D + (h0 + HPC) * HD]]
        w_cat = np.ascontiguousarray(np.concatenate(cols, axis=1),
                                     dtype=np.float32)          # [D, 768]
        w_o = np.ascontiguousarray(
            w_out[h0 * HD:(h0 + HPC) * HD, :]).astype(bf)       # [256, D]
        in_maps.append({
            "xT": xT.astype(np.float32, copy=False),
            "w_cat": w_cat,
            "w_o": w_o,
            "cos2": cos2d,
            "sin2": sin2d,
        })
    return in_maps


def kernel(x, w_qkv, w_out):
    x = np.asarray(x, dtype=np.float32)
    w_qkv = np.asarray(w_qkv, dtype=np.float32)
    w_out = np.asarray(w_out, dtype=np.float32)
    nc = _get_program()
    in_maps = _host_prep(x, w_qkv, w_out)
    res = run_bass_kernel_spmd(nc, in_maps, list(range(NCORES)), trace=False)
    out = np.zeros((B, T, D), dtype=np.float32)
    for core in range(NCORES):
        b = core // GROUPS
        out[b] += res.results[core]["outp"].T.astype(np.float32)
        # strip-2 pair-0 oproj partial shipped separately
        out[b][2 * 512:3 * 512] += \
            res.results[core]["outp2"].T.astype(np.float32)
    return out
